# revision 9
# baseline (speedup 1.0000x reference)
"""Bahdanau additive attention on 8 Trainium2 NeuronCores.

Problem shapes (hardcoded): B=4, T=128, S=512, H=256, fp32.

Sharding: data-parallel over (batch, T-half): core c handles b = c//2,
t in [64*(c%2), 64*(c%2)+64).  Every core runs the same SPMD program on
its own shard; weights are replicated.  No collectives.

Per-core algorithm (T_loc=64, S=512, H=256):
  peT[h,s] = (Wh @ enc^T)[h,s]        f32r matmuls
  pqT[h,t] = (Ws @ q^T)[h,t]          fp32 matmuls
  For each t:  Y[h,s] = peT[h,s] + pqT[h,t]   (DVE tensor_scalar_add,
               pq column is the per-partition scalar operand)
  X = tanh(Y)                          ACT on big (128 x 4096) tiles
  e[t,s] = sum_h v[h]*X[h,s]           PE, v stationary (128x1), f32r
  P = exp(e)                           no max-subtraction: |e| <= ||v||_1
  PT = P^T (PE transpose), PTm = PT * mask[s]  (per-partition mul)
  Z[t] = sum_s PTm[s,t]*mask[s]        PE matmul, out (64x1) t-on-partition
  c[t,:] = (1/Z[t]) * sum_s PTm[s,t]*enc[s,:]
  attn = tanh([q,c] @ Wout^T)          catT = [qT; cT], f32r matmuls
"""

import numpy as np

B, T, S, H = 4, 128, 512, 256
TLOC = 64          # T rows per core
NCORES = 8
TGS = 8            # t's per tanh group
NG = TLOC // TGS   # 8 groups
P = 128            # partitions
HC = H // P        # 2 h-chunks
SB = S // P        # 4 s-blocks
FC = (2 * H) // P  # 4 f-chunks of cat=[q,c]

_CACHE = {}


def build_module():
    """Build + compile the SPMD Bass module (same program for all cores)."""
    if "nc" in _CACHE:
        return _CACHE["nc"]

    import concourse.bass as bass
    import concourse.tile as tile
    from concourse import bacc, mybir

    f32 = mybir.dt.float32
    f32r = mybir.dt.float32r
    AF = mybir.ActivationFunctionType

    nc = bacc.Bacc(
        "TRN2",
        target_bir_lowering=False,
        debug=False,
        enable_asserts=False,
        num_devices=NCORES,
    )

    d_qT = nc.dram_tensor("qT_l", (H, TLOC), f32, kind="ExternalInput").ap()
    d_encT = nc.dram_tensor("encT_l", (H, S), f32, kind="ExternalInput").ap()
    d_enc = nc.dram_tensor("enc_l", (S, H), f32, kind="ExternalInput").ap()
    d_wsT = nc.dram_tensor("wsT", (H, H), f32, kind="ExternalInput").ap()
    d_whT = nc.dram_tensor("whT", (H, H), f32, kind="ExternalInput").ap()
    d_woutT = nc.dram_tensor("woutT", (2 * H, H), f32, kind="ExternalInput").ap()
    # vz[hc]: (128 x 128) with v[hc*128:(hc+1)*128] in column 64, zeros
    # elsewhere.  lhsT = vz[:, 64-t:128-t] places v in column t, so a single
    # M=64 matmul writes row t = v^T X_t and zeros elsewhere (PSUM-accumulated
    # across all t).  Works around the PE's base-partition-0/32/64 restriction
    # on outputs at the cost of nothing (matmul time scales with N only).
    d_vz = nc.dram_tensor("vz", (HC * P, P), f32, kind="ExternalInput").ap()
    d_mask = nc.dram_tensor("maskc", (P, SB), f32, kind="ExternalInput").ap()
    d_ident = nc.dram_tensor("ident", (TLOC, TLOC), f32, kind="ExternalInput").ap()
    d_out = nc.dram_tensor("out_l", (TLOC, H), f32, kind="ExternalOutput").ap()

    with tile.TileContext(nc) as tc:
        from contextlib import ExitStack

        with ExitStack() as ctx:
            consts = ctx.enter_context(tc.tile_pool(name="consts", bufs=1))
            proj = ctx.enter_context(tc.tile_pool(name="proj", bufs=1))
            ypool = ctx.enter_context(tc.tile_pool(name="ypool", bufs=2))
            xpool = ctx.enter_context(tc.tile_pool(name="xpool", bufs=2))
            tail = ctx.enter_context(tc.tile_pool(name="tail", bufs=1))
            psA = ctx.enter_context(tc.tile_pool(name="psA", bufs=2, space="PSUM"))
            psE = ctx.enter_context(tc.tile_pool(name="psE", bufs=1, space="PSUM"))
            psT = ctx.enter_context(tc.tile_pool(name="psT", bufs=3, space="PSUM"))

            # ---- load constants / inputs ----
            ws_sb = []
            wh_sb = []
            wout_sb = []
            qT_sb = []
            encT_sb = []
            enc_sb = []
            for kc in range(HC):
                t1 = consts.tile([P, H], f32, name=f"ws_sb{kc}")
                nc.sync.dma_start(t1[:], d_wsT[kc * P:(kc + 1) * P, :])
                ws_sb.append(t1)
                t2 = consts.tile([P, H], f32, name=f"wh_sb{kc}")
                nc.sync.dma_start(t2[:], d_whT[kc * P:(kc + 1) * P, :])
                wh_sb.append(t2)
                t3 = consts.tile([P, TLOC], f32, name=f"qT_sb{kc}")
                nc.sync.dma_start(t3[:], d_qT[kc * P:(kc + 1) * P, :])
                qT_sb.append(t3)
                t4 = consts.tile([P, S], f32, name=f"encT_sb{kc}")
                nc.sync.dma_start(t4[:], d_encT[kc * P:(kc + 1) * P, :])
                encT_sb.append(t4)
            for fc in range(FC):
                t5 = consts.tile([P, H], f32, name=f"wout_sb{fc}")
                nc.sync.dma_start(t5[:], d_woutT[fc * P:(fc + 1) * P, :])
                wout_sb.append(t5)
            for sb in range(SB):
                t6 = consts.tile([P, H], f32, name=f"enc_sb{sb}")
                nc.sync.dma_start(t6[:], d_enc[sb * P:(sb + 1) * P, :])
                enc_sb.append(t6)
            vz_sb = []
            for hc in range(HC):
                t6b = consts.tile([P, P], f32, name=f"vz_sb{hc}")
                nc.sync.dma_start(t6b[:], d_vz[hc * P:(hc + 1) * P, :])
                # walrus requires f32r matmul operands to be produced as
                # f32r; a DVE copy is the rounding producer
                t6r = consts.tile([P, P], f32r, name=f"vzr_sb{hc}")
                nc.vector.tensor_copy(t6r[:], t6b[:])
                vz_sb.append(t6r)
            mask_sb = consts.tile([P, SB], f32)
            nc.sync.dma_start(mask_sb[:], d_mask[:, :])
            ident_sb = consts.tile([TLOC, TLOC], f32)
            nc.sync.dma_start(ident_sb[:], d_ident[:, :])

            # ---- projections ----
            # peT[oc] (128 x 512): peT[o,s] = sum_h Wh[o,h] * encT[h,s]
            peT_sb = []
            for oc in range(HC):
                pe_ps = psA.tile([P, S], f32, name=f"pe_ps{oc}", tag="pe_ps")
                for kc in range(HC):
                    nc.tensor.matmul(
                        pe_ps[:],
                        lhsT=wh_sb[kc][:, oc * P:(oc + 1) * P],
                        rhs=encT_sb[kc][:],
                        start=(kc == 0),
                        stop=(kc == HC - 1),
                    )
                t7 = proj.tile([P, S], f32, name=f"peT_sb{oc}")
                nc.vector.tensor_copy(t7[:], pe_ps[:])
                peT_sb.append(t7)

            # pqT[oc] (128 x 64): pqT[o,t] = sum_h Ws[o,h] * qT[h,t]  (fp32)
            pqT_sb = []
            for oc in range(HC):
                pq_ps = psT.tile([P, TLOC], f32, name=f"pq_ps{oc}", tag="tail")
                for kc in range(HC):
                    nc.tensor.matmul(
                        pq_ps[:],
                        lhsT=ws_sb[kc][:, oc * P:(oc + 1) * P],
                        rhs=qT_sb[kc][:],
                        start=(kc == 0),
                        stop=(kc == HC - 1),
                    )
                t8 = proj.tile([P, TLOC], f32, name=f"pqT_sb{oc}")
                nc.vector.tensor_copy(t8[:], pq_ps[:])
                pqT_sb.append(t8)

            # ---- main loop: Y = pe + pq_t ; X = tanh(Y) ; e = v^T X ----
            e_ps = psE.tile([TLOC, S], f32)
            for g in range(NG):
                for hc in range(HC):
                    y = ypool.tile([P, TGS * S], f32, name=f"y_{g}_{hc}",
                                   tag=f"y{hc}")
                    for j in range(TGS):
                        t = g * TGS + j
                        nc.vector.tensor_scalar_add(
                            y[:, j * S:(j + 1) * S],
                            peT_sb[hc][:],
                            pqT_sb[hc][:, t:t + 1],
                        )
                    x = xpool.tile([P, TGS * S], f32r, name=f"x_{g}_{hc}",
                                   tag=f"x{hc}")
                    nc.scalar.activation(x[:], y[:], AF.Tanh)
                    for j in range(TGS):
                        t = g * TGS + j
                        first = (g == 0 and hc == 0 and j == 0)
                        last = (g == NG - 1 and hc == HC - 1 and j == TGS - 1)
                        nc.tensor.matmul(
                            e_ps[:, :],
                            lhsT=vz_sb[hc][:, TLOC - t:2 * TLOC - t],
                            rhs=x[:, j * S:(j + 1) * S],
                            start=first,
                            stop=last,
                            skip_group_check=True,
                        )

            # ---- softmax tail ----
            p_sb = tail.tile([TLOC, S], f32)
            nc.scalar.activation(p_sb[:], e_ps[:], AF.Exp)

            pt_ps = psT.tile([P, SB * TLOC], f32, tag="tail")
            for sb in range(SB):
                nc.tensor.transpose(
                    pt_ps[:, sb * TLOC:(sb + 1) * TLOC],
                    p_sb[:, sb * P:(sb + 1) * P],
                    ident_sb[:],
                )
            ptm_sb = []
            for sb in range(SB):
                t9 = tail.tile([P, TLOC], f32, name=f"ptm_sb{sb}")
                nc.vector.tensor_scalar_mul(
                    t9[:],
                    pt_ps[:, sb * TLOC:(sb + 1) * TLOC],
                    mask_sb[:, sb:sb + 1],
                )
                ptm_sb.append(t9)

            z_ps = psT.tile([TLOC, 1], f32, tag="tail")
            for sb in range(SB):
                nc.tensor.matmul(
                    z_ps[:],
                    lhsT=ptm_sb[sb][:],
                    rhs=mask_sb[:, sb:sb + 1],
                    start=(sb == 0),
                    stop=(sb == SB - 1),
                )
            r_sb = tail.tile([TLOC, 1], f32)
            nc.vector.reciprocal(r_sb[:], z_ps[:])

            cun_ps = psT.tile([TLOC, H], f32, tag="tail")
            for sb in range(SB):
                nc.tensor.matmul(
                    cun_ps[:],
                    lhsT=ptm_sb[sb][:],
                    rhs=enc_sb[sb][:],
                    start=(sb == 0),
                    stop=(sb == SB - 1),
                )
            c_sb = tail.tile([TLOC, H], f32)
            nc.vector.tensor_scalar_mul(c_sb[:], cun_ps[:], r_sb[:])

            ct_ps = psT.tile([P, 2 * TLOC], f32, tag="tail")
            for i in range(HC):
                nc.tensor.transpose(
                    ct_ps[:, i * TLOC:(i + 1) * TLOC],
                    c_sb[:, i * P:(i + 1) * P],
                    ident_sb[:],
                )
            ct_sb = tail.tile([P, 2 * TLOC], f32)
            nc.vector.tensor_copy(ct_sb[:], ct_ps[:])

            attn_ps = psT.tile([TLOC, H], f32, tag="tail")
            cat_tiles = [
                qT_sb[0][:],
                qT_sb[1][:],
                ct_sb[:, 0:TLOC],
                ct_sb[:, TLOC:2 * TLOC],
            ]
            for fc in range(FC):
                nc.tensor.matmul(
                    attn_ps[:],
                    lhsT=cat_tiles[fc],
                    rhs=wout_sb[fc][:],
                    start=(fc == 0),
                    stop=(fc == FC - 1),
                )
            o_sb = tail.tile([TLOC, H], f32)
            nc.scalar.activation(o_sb[:], attn_ps[:], AF.Tanh)
            nc.sync.dma_start(d_out[:, :], o_sb[:])

    nc.compile()
    _CACHE["nc"] = nc
    return nc


def make_in_maps(query, encoder_outputs, src_lengths, Ws, Wh, v, Wout):
    """Host-side shard/layout prep: per-core input dict (all fp32, C-order)."""
    f = np.float32
    wsT = np.ascontiguousarray(np.asarray(Ws, f).T)
    whT = np.ascontiguousarray(np.asarray(Wh, f).T)
    woutT = np.ascontiguousarray(np.asarray(Wout, f).T)
    vz = np.zeros((HC * P, P), f)
    for hc in range(HC):
        vz[hc * P:(hc + 1) * P, TLOC] = np.asarray(v, f)[hc * P:(hc + 1) * P]
    ident = np.eye(TLOC, dtype=f)
    sl = np.asarray(src_lengths)
    in_maps = []
    for c in range(NCORES):
        b, th = c // 2, c % 2
        t0 = th * TLOC
        maskc = (np.arange(S)[:, None] < int(sl[b])).astype(f)  # (S,1)
        maskc = np.ascontiguousarray(maskc.reshape(SB, P).T)    # (128,4)
        in_maps.append({
            "qT_l": np.ascontiguousarray(
                np.asarray(query[b, t0:t0 + TLOC, :], f).T),
            "encT_l": np.ascontiguousarray(
                np.asarray(encoder_outputs[b], f).T),
            "enc_l": np.ascontiguousarray(np.asarray(encoder_outputs[b], f)),
            "wsT": wsT,
            "whT": whT,
            "woutT": woutT,
            "vz": vz,
            "maskc": maskc,
            "ident": ident,
        })
    return in_maps


def kernel(query, encoder_outputs, src_lengths, Ws, Wh, v, Wout):
    from concourse.bass_utils import run_bass_kernel_spmd

    nc = build_module()
    in_maps = make_in_maps(query, encoder_outputs, src_lengths, Ws, Wh, v, Wout)
    res = run_bass_kernel_spmd(nc, in_maps, core_ids=list(range(NCORES))).results
    out = np.empty((B, T, H), np.float32)
    for c in range(NCORES):
        b, th = c // 2, c % 2
        t0 = th * TLOC
        out[b, t0:t0 + TLOC, :] = res[c]["out_l"]
    return out
